# revision 1
# baseline (speedup 1.0000x reference)
"""Multi-head causal self-attention block on 8 Trainium2 NeuronCores.

Reference computation (fp32):
    qkv = x @ W1.T + b1          x:(2,2048,768)  W1:(2304,768)
    q,k,v split -> 12 heads of 64
    scores = causal(q @ k.T / 8), softmax, o = attn @ v
    out = o @ W2.T + b2

Sharding: core = batch b (2) x head-group g (4, 3 heads each).
Each core computes QKV for its heads (TP columns of W1), attention, and a
partial out-projection over its 192 channels (TP rows of W2).  Host sums the
4 partials per batch (the TP all-reduce) and adds b2.

Device kernel design:
  - activations kept transposed: xT (c, t); q/k as qT,kT (64, 2048);
    scores computed key-major sT[tk, m] so exp(sT) feeds the PV matmul with
    the contraction dim (tk) on partitions -- no on-chip transposes at all.
  - fully pipelined by query m-block i: x arrives in 4 t-quarters (separate
    DMA tensors), each iteration projects quarter i (qk + v), runs causal
    attention for m-block i, and emits the partial out-projection for i.
  - no softmax max-subtraction: logits are ~N(0,1) (max |logit| << 88).
  - softmax denominator: ones column appended to v (row 64 of the PV psum);
    the ones are produced by the matmul itself (aug row x indicator column).
  - biases: ones row appended to xT, bias row appended to the weights.
  - causal masking on diagonal tiles: either an extra accumulating matmul
    (identity.T @ additive mask, PE) or a post-exp binary multiply (GPSIMD).
  - all matmuls are float32r (full PE rate at N>=256 vs 1/4 for fp32).
"""

import os

import numpy as np

import concourse.bass as bass
import concourse.tile as tile
from concourse import bacc
from concourse import mybir
from concourse import bass_utils

B = 2
T = 2048
C = 768
NH = 12
D = 64
NCORES = 8
GROUPS = 4               # head groups (tensor parallel)
NH_CORE = NH // GROUPS   # 3 heads per core
CC = NH_CORE * D         # 192 channels per core
MB = 512                 # query m-block width (PSUM bank)
NMB = T // MB            # 4 m-blocks
NTK = T // 128           # 16 key tiles
VW = D + 1               # v with ones column
NCA = C + 1              # contraction rows incl. bias/ones row
NCT = 7                  # c-tiles (6x128 + 1x1)
F32 = mybir.dt.float32
F32R = mybir.dt.float32r
MASK_VAL = -1.0e9

# packed qvx0 column layout: [w1qk | w1v | x quarter 0]
QKW = 2 * CC             # 384: qk weights
VWD = 256                # v weights (192 used + ones col at 192)
XOF = QKW + VWD          # 640: x quarter 0 starts here
Q0W = XOF + MB           # 1152

# mi layout: [additive masks | identity | binary masks]
MIW = 4 * MB + 128 + 4 * MB

LAST_RESULTS = None      # BassKernelResults of the last run (for test.py)


def _flag(name, default):
    return int(os.environ.get(name, default))


def _build_masks() -> np.ndarray:
    """[128, MIW]: 4 additive causal tiles, 128x128 identity, 4 binary tiles.

    tile p covers keys t = 128p + row vs queries m = col (within an m-block).
    """
    out = np.zeros((128, MIW), np.float32)
    m = np.arange(MB)[None, :]
    for p in range(4):
        t = 128 * p + np.arange(128)[:, None]
        keep = t <= m
        out[:, MB * p : MB * (p + 1)] = np.where(keep, 0.0, MASK_VAL)
        out[:, 4 * MB + 128 + MB * p : 4 * MB + 128 + MB * (p + 1)] = keep
    out[:, 4 * MB : 4 * MB + 128] = np.eye(128, dtype=np.float32)
    return out


def _build_program() -> bass.Bass:
    mask_on_pe = _flag("K_MASK_PE", 1)
    qk_bufs = _flag("K_QK_BUFS", 2)
    pv_bufs = _flag("K_PV_BUFS", 2)
    proj_bufs = _flag("K_PROJ_BUFS", 2)
    pt_bufs = _flag("K_PT_BUFS", 3)

    nc = bacc.Bacc(
        "TRN2", target_bir_lowering=False, debug=False, num_devices=NCORES
    )

    q_d = [
        nc.dram_tensor("qvx0", (NCA, Q0W), F32R, kind="ExternalInput").ap(),
        nc.dram_tensor("qvx1", (NCA, MB), F32R, kind="ExternalInput").ap(),
        nc.dram_tensor("qvx2", (NCA, MB), F32R, kind="ExternalInput").ap(),
        nc.dram_tensor("qvx3", (NCA, MB), F32R, kind="ExternalInput").ap(),
    ]
    w2_d = nc.dram_tensor("w2T", (CC, C), F32R, kind="ExternalInput").ap()
    mi_d = nc.dram_tensor("mi", (128, MIW), F32R, kind="ExternalInput").ap()
    out_d = nc.dram_tensor("outT", (C, T), F32, kind="ExternalOutput").ap()

    with tile.TileContext(nc) as tc:
        with (
            nc.allow_low_precision(reason="float32r tensors for PE-rate matmuls"),
            tc.tile_pool(name="persist", bufs=1) as persist,
            tc.tile_pool(name="pt_pool", bufs=pt_bufs) as pt_pool,
            tc.tile_pool(name="small", bufs=2) as small,
            tc.tile_pool(name="ostage", bufs=2) as ostage,
            tc.tile_pool(name="proj_ps", bufs=proj_bufs, space="PSUM") as proj_ps,
            tc.tile_pool(name="qk_ps", bufs=qk_bufs, space="PSUM") as qk_ps,
            tc.tile_pool(name="pv_ps", bufs=pv_bufs, space="PSUM") as pv_ps,
        ):
            # ---- input DMAs: quarter 0 (+weights) first, then the rest ----
            qv = [[None] * NCT for _ in range(4)]
            for ci in range(NCT):
                p = 128 if ci < NCT - 1 else NCA - 128 * (NCT - 1)
                t0 = persist.tile([p, Q0W], F32R, tag=f"q0_{ci}")
                nc.sync.dma_start(t0, q_d[0][128 * ci : 128 * ci + p, :])
                qv[0][ci] = t0
            mi = persist.tile([128, MIW], F32R, tag="mi")
            nc.sync.dma_start(mi, mi_d)
            w2a = persist.tile([128, C], F32R, tag="w2a")
            nc.sync.dma_start(w2a, w2_d[0:128, :])
            w2b = persist.tile([CC - 128, C], F32R, tag="w2b")
            nc.sync.dma_start(w2b, w2_d[128:CC, :])
            for q in range(1, 4):
                for ci in range(NCT):
                    p = 128 if ci < NCT - 1 else NCA - 128 * (NCT - 1)
                    tq = persist.tile([p, MB], F32R, tag=f"q{q}_{ci}")
                    nc.sync.dma_start(tq, q_d[q][128 * ci : 128 * ci + p, :])
                    qv[q][ci] = tq
            def xap(ci, q):
                """x columns for t-quarter q on c-tile ci (p, 512)."""
                if q == 0:
                    return qv[0][ci][:, XOF:Q0W]
                return qv[q][ci]

            add_mask = lambda p: mi[:, MB * p : MB * (p + 1)]
            ident = mi[:, 4 * MB : 4 * MB + 128]
            bin_mask = lambda p: mi[:, 4 * MB + 128 + MB * p : 4 * MB + 128 + MB * (p + 1)]

            v_sb = persist.tile([128, NTK * NH_CORE * VW], F32R, tag="v_sb")
            qT = []
            kT = []
            for hh in range(NH_CORE):
                qT.append(
                    persist.tile([D, T], F32R, tag=f"qT{hh}", name=f"qT{hh}")
                )
                kT.append(
                    persist.tile([D, T], F32R, tag=f"kT{hh}", name=f"kT{hh}")
                )
            oT_a = persist.tile([128, T], F32R, tag="oT_a")  # heads 0,1
            oT_b = persist.tile([D, T], F32R, tag="oT_b")    # head 2

            for i in range(NMB):
                # ---- qk projection for t-quarter i ----
                for hh in range(NH_CORE):
                    ps = proj_ps.tile([128, MB], F32, tag="ps")
                    for ci in range(NCT):
                        nc.tensor.matmul(
                            ps,
                            lhsT=qv[0][ci][:, 128 * hh : 128 * hh + 128],
                            rhs=xap(ci, i),
                            start=(ci == 0),
                            stop=(ci == NCT - 1),
                        )
                    nc.vector.tensor_copy(
                        qT[hh][:, MB * i : MB * (i + 1)], ps[0:D, :]
                    )
                    nc.vector.tensor_copy(
                        kT[hh][:, MB * i : MB * (i + 1)], ps[D:128, :]
                    )
                # ---- v projection for t-chunks 4i..4i+3 ----
                for tch in range(4 * i, 4 * i + 4):
                    ps = proj_ps.tile([128, VWD], F32, tag="ps")
                    for ci in range(NCT):
                        nc.tensor.matmul(
                            ps,
                            lhsT=xap(ci, i)[:, 128 * (tch % 4) : 128 * (tch % 4 + 1)],
                            rhs=qv[0][ci][:, QKW:XOF],
                            start=(ci == 0),
                            stop=(ci == NCT - 1),
                        )
                    chunk = v_sb[
                        :, NH_CORE * VW * tch : NH_CORE * VW * (tch + 1)
                    ].rearrange("p (h u) -> p h u", h=NH_CORE)
                    nc.vector.tensor_copy(
                        chunk[:, :, 0:D],
                        ps[:, 0:CC].rearrange("p (h u) -> p h u", h=NH_CORE),
                    )
                    nc.vector.tensor_copy(
                        chunk[:, :, D : D + 1].squeeze(),
                        ps[:, CC : CC + 1].broadcast_to((128, NH_CORE)),
                    )

                # ---- attention for m-block i ----
                for hh in range(NH_CORE):
                    pvps = pv_ps.tile([128, MB], F32, tag="pv")
                    njt = 4 * (i + 1)  # key tiles needed (always even)
                    for j0 in range(0, njt, 2):
                        qkps = qk_ps.tile([128, 2 * MB], F32, tag="qk")
                        for u in range(2):
                            j = j0 + u
                            diag = j >= 4 * i
                            half = qkps[:, MB * u : MB * (u + 1)]
                            nc.tensor.matmul(
                                half,
                                lhsT=kT[hh][:, 128 * j : 128 * (j + 1)],
                                rhs=qT[hh][:, MB * i : MB * (i + 1)],
                                start=True,
                                stop=not (diag and mask_on_pe),
                            )
                            if diag and mask_on_pe:
                                nc.tensor.matmul(
                                    half,
                                    lhsT=ident,
                                    rhs=add_mask(j - 4 * i),
                                    start=False,
                                    stop=True,
                                )
                        pt = pt_pool.tile([128, 2 * MB], F32R, tag="pt")
                        nc.scalar.activation(
                            pt, qkps, mybir.ActivationFunctionType.Exp
                        )
                        for u in range(2):
                            j = j0 + u
                            if j >= 4 * i and not mask_on_pe:
                                nc.gpsimd.tensor_mul(
                                    pt[:, MB * u : MB * (u + 1)],
                                    pt[:, MB * u : MB * (u + 1)],
                                    bin_mask(j - 4 * i),
                                )
                            vj = v_sb[
                                :,
                                NH_CORE * VW * j + VW * hh :
                                NH_CORE * VW * j + VW * hh + VW,
                            ]
                            nc.tensor.matmul(
                                pvps[0:VW, :],
                                lhsT=vj,
                                rhs=pt[:, MB * u : MB * (u + 1)],
                                start=(j == 0),
                                stop=(j == njt - 1),
                            )
                    # normalize: o = pv[0:64] / pv[64]
                    rrow = small.tile([1, MB], F32, tag="rrow")
                    nc.vector.reciprocal(rrow, pvps[D : D + 1, :])
                    rbc = small.tile([D, MB], F32, tag="rbc")
                    nc.gpsimd.partition_broadcast(rbc, rrow)
                    if hh < 2:
                        odst = oT_a[D * hh : D * (hh + 1), MB * i : MB * (i + 1)]
                    else:
                        odst = oT_b[:, MB * i : MB * (i + 1)]
                    nc.vector.tensor_mul(odst, pvps[0:D, :], rbc)

                # ---- partial output projection for m-block i ----
                for fc in range(C // 128):
                    ps = proj_ps.tile([128, MB], F32, tag="ps")
                    nc.tensor.matmul(
                        ps,
                        lhsT=w2a[:, 128 * fc : 128 * (fc + 1)],
                        rhs=oT_a[:, MB * i : MB * (i + 1)],
                        start=True,
                        stop=False,
                    )
                    nc.tensor.matmul(
                        ps,
                        lhsT=w2b[:, 128 * fc : 128 * (fc + 1)],
                        rhs=oT_b[:, MB * i : MB * (i + 1)],
                        start=False,
                        stop=True,
                    )
                    osb = ostage.tile([128, MB], F32, tag="osb")
                    nc.vector.tensor_copy(osb, ps)
                    nc.sync.dma_start(
                        out_d[128 * fc : 128 * (fc + 1), MB * i : MB * (i + 1)],
                        osb,
                    )
    nc.compile()
    return nc


GROUPS_HEADS = [[3 * g + k for k in range(NH_CORE)] for g in range(GROUPS)]


def _prep_core_inputs(x, W1, b1, W2):
    """Per-core input dicts. Core index = 4*b + g."""
    mi = _build_masks()
    scale = np.float32(1.0 / np.sqrt(D))  # 1/8, exact in fp32
    in_maps = []
    for b in range(B):
        xT = np.concatenate(
            [np.asarray(x[b]).T, np.ones((1, T), np.float32)], axis=0
        )  # (769, 2048)
        for g in range(GROUPS):
            heads = GROUPS_HEADS[g]
            q0 = np.zeros((NCA, Q0W), np.float32)
            # qk weights: per head [q(64) scaled | k(64)], bias in aug row
            for hh, h in enumerate(heads):
                q0[:C, 128 * hh : 128 * hh + D] = (W1[D * h : D * h + D] * scale).T
                q0[C, 128 * hh : 128 * hh + D] = b1[D * h : D * h + D] * scale
                q0[:C, 128 * hh + D : 128 * hh + 128] = W1[
                    C + D * h : C + D * h + D
                ].T
                q0[C, 128 * hh + D : 128 * hh + 128] = b1[C + D * h : C + D * h + D]
                q0[:C, QKW + D * hh : QKW + D * hh + D] = W1[
                    2 * C + D * h : 2 * C + D * h + D
                ].T
                q0[C, QKW + D * hh : QKW + D * hh + D] = b1[
                    2 * C + D * h : 2 * C + D * h + D
                ]
            q0[C, QKW + CC] = 1.0  # ones-producer column for v
            q0[:, XOF:] = xT[:, 0:MB]
            # out-proj rows for this core's channels
            w2T = np.empty((CC, C), np.float32)
            for hh, h in enumerate(heads):
                w2T[D * hh : D * hh + D] = W2[:, D * h : D * h + D].T
            in_maps.append(
                {
                    "qvx0": q0,
                    "qvx1": np.ascontiguousarray(xT[:, MB : 2 * MB]),
                    "qvx2": np.ascontiguousarray(xT[:, 2 * MB : 3 * MB]),
                    "qvx3": np.ascontiguousarray(xT[:, 3 * MB : 4 * MB]),
                    "w2T": np.ascontiguousarray(w2T),
                    "mi": mi,
                }
            )
    return in_maps


_PROGRAM_CACHE = {}


def kernel(x, W1, b1, W2, b2):
    global LAST_RESULTS
    x = np.asarray(x, np.float32)
    W1 = np.asarray(W1, np.float32)
    b1 = np.asarray(b1, np.float32)
    W2 = np.asarray(W2, np.float32)
    b2 = np.asarray(b2, np.float32)

    if "prog" not in _PROGRAM_CACHE:
        _PROGRAM_CACHE["prog"] = _build_program()
    nc = _PROGRAM_CACHE["prog"]

    in_maps = _prep_core_inputs(x, W1, b1, W2)
    trace = os.environ.get("KERNEL_TRACE", "0") == "1"
    res = bass_utils.run_bass_kernel_spmd(
        nc, in_maps, core_ids=list(range(NCORES)), trace=trace
    )
    LAST_RESULTS = res

    out = np.empty((B, T, C), np.float32)
    for b in range(B):
        acc = res.results[GROUPS * b]["outT"].astype(np.float32).copy()
        for g in range(1, GROUPS):
            acc += res.results[GROUPS * b + g]["outT"]
        out[b] = acc.T + b2[None, :]
    return out



# revision 10
# speedup vs baseline: 2.0723x; 2.0723x over previous
"""Multi-head causal self-attention block on 8 Trainium2 NeuronCores.

Reference computation (fp32):
    qkv = x @ W1.T + b1          x:(2,2048,768)  W1:(2304,768)
    q,k,v split -> 12 heads of 64
    scores = causal(q @ k.T / 8), softmax, o = attn @ v
    out = o @ W2.T + b2

Sharding: core = batch b (2) x head-group g (4, 3 heads each).
Each core computes QKV for its heads (TP columns of W1), attention, and a
partial out-projection over its 192 channels (TP rows of W2).  Host sums the
4 partials per batch (the TP all-reduce) and adds b2' = b2 + W2 @ b1_v
(the v-bias is linear through attention since softmax weights sum to 1).

v2 design (bf16 everywhere on the PE):
  - all matmul operands bf16 (PSUM accumulates fp32): sustains ~260ns per
    512-col matmul and, unlike float32r, has no 4x penalty below N=256,
    which unlocks narrow diagonal tiles.
  - contraction is exactly 768 = 6 c-tiles (no bias row): q/k bias is added
    per-partition during the PSUM->SBUF copy (tensor_scalar_add), the
    softmax-denominator ones column is memset once.
  - q|k kept packed per head in one [128, T] tile -> single copy per
    (head, m-block).
  - diagonal m-blocks narrowed: for diag key-tile p only query columns
    >= 128p are computed (QK, exp, PV), and the causal mask is one shared
    [128,128] lower-triangular additive matmul on the mixed block.
  - softmax denominator reciprocal via reciprocal_approx_fast (~0.8us vs
    4us for vector.reciprocal, which serialized the out-projection).
  - out-projection for block i is emitted in iteration i+1 (before that
    block's attention) so it never waits on the normalize chain.
"""

import os

import numpy as np
import ml_dtypes

import concourse.bass as bass
import concourse.tile as tile
from concourse import bacc
from concourse import mybir
from concourse import bass_utils

B = 2
T = 2048
C = 768
NH = 12
D = 64
NCORES = 8
GROUPS = 4               # head groups (tensor parallel)
NH_CORE = NH // GROUPS   # 3 heads per core
CC = NH_CORE * D         # 192 channels per core
MB = 512                 # query m-block width (PSUM bank)
NMB = T // MB            # 4 m-blocks
NTK = T // 128           # 16 key tiles
VW = D + 1               # v with ones column
NCT = C // 128           # 6 c-tiles
F32 = mybir.dt.float32
BF16 = mybir.dt.bfloat16
MASK_VAL = -1.0e9

LAST_RESULTS = None      # BassKernelResults of the last run (for test.py)


def _flag(name, default):
    return int(os.environ.get(name, default))


def _build_masks() -> np.ndarray:
    """[128, 256] bf16: lower-tri additive mask (keep iff key<=query) | identity."""
    out = np.zeros((128, 256), np.float32)
    r = np.arange(128)[:, None]
    c = np.arange(128)[None, :]
    out[:, 0:128] = np.where(r <= c, 0.0, MASK_VAL)
    out[:, 128:256] = np.eye(128, dtype=np.float32)
    return out.astype(ml_dtypes.bfloat16)


def _build_program() -> bass.Bass:
    pt_bufs = _flag("K_PT_BUFS", 3)
    qk_bufs = _flag("K_QK_BUFS", 2)
    pv_bufs = _flag("K_PV_BUFS", 2)
    proj_bufs = _flag("K_PROJ_BUFS", 2)

    nc = bacc.Bacc(
        "TRN2", target_bir_lowering=False, debug=False, num_devices=NCORES
    )

    debug = _flag("K_DEBUG", 0)
    x_d = nc.dram_tensor("x", (C, T), BF16, kind="ExternalInput").ap()
    w1qk_d = nc.dram_tensor("w1qk", (C, 2 * CC), BF16, kind="ExternalInput").ap()
    w1v_d = nc.dram_tensor("w1v", (C, CC), BF16, kind="ExternalInput").ap()
    w2a_d = nc.dram_tensor("w2a", (128, C), BF16, kind="ExternalInput").ap()
    w2b_d = nc.dram_tensor("w2b", (CC - 128, C), BF16, kind="ExternalInput").ap()
    mi_d = nc.dram_tensor("mi", (128, 256), BF16, kind="ExternalInput").ap()
    bqk_d = nc.dram_tensor("bqk", (128, NH_CORE), F32, kind="ExternalInput").ap()
    out_d = nc.dram_tensor("outT", (C, T), BF16, kind="ExternalOutput").ap()
    if debug:
        v_dbg = nc.dram_tensor(
            "v_dbg", (128, NTK * NH_CORE * VW), BF16, kind="ExternalOutput"
        ).ap()
        q_dbg = nc.dram_tensor("q_dbg", (D, T), BF16, kind="ExternalOutput").ap()
        k_dbg = nc.dram_tensor("k_dbg", (D, T), BF16, kind="ExternalOutput").ap()
        oa_dbg = nc.dram_tensor("oa_dbg", (128, T), BF16, kind="ExternalOutput").ap()

    with tile.TileContext(nc) as tc:
        with (
            nc.allow_low_precision(reason="bf16 matmuls within 2e-2 tolerance"),
            tc.tile_pool(name="persist", bufs=1) as persist,
            tc.tile_pool(name="pt_pool", bufs=pt_bufs) as pt_pool,
            tc.tile_pool(name="small", bufs=2) as small,
            tc.tile_pool(name="ostage", bufs=2) as ostage,
            tc.tile_pool(name="proj_ps", bufs=proj_bufs, space="PSUM") as proj_ps,
            tc.tile_pool(name="qk_ps", bufs=qk_bufs, space="PSUM") as qk_ps,
            tc.tile_pool(name="pv_ps", bufs=pv_bufs, space="PSUM") as pv_ps,
        ):
            # ---- input DMAs: x quarter 0 + weights first, then quarters 1-3
            xt = [[None] * NCT for _ in range(NMB)]
            for ci in range(NCT):
                t0 = persist.tile([128, MB], BF16, tag=f"x0_{ci}")
                nc.sync.dma_start(t0, x_d[128 * ci : 128 * (ci + 1), 0:MB])
                xt[0][ci] = t0
            w1qk = []
            w1v = []
            for ci in range(NCT):
                wt = persist.tile([128, 2 * CC], BF16, tag=f"w1qk_{ci}")
                nc.sync.dma_start(wt, w1qk_d[128 * ci : 128 * (ci + 1), :])
                w1qk.append(wt)
                vt = persist.tile([128, CC], BF16, tag=f"w1v_{ci}")
                nc.sync.dma_start(vt, w1v_d[128 * ci : 128 * (ci + 1), :])
                w1v.append(vt)
            mi = persist.tile([128, 256], BF16, tag="mi")
            nc.sync.dma_start(mi, mi_d)
            bqk = persist.tile([128, NH_CORE], F32, tag="bqk")
            nc.sync.dma_start(bqk, bqk_d)
            w2a = persist.tile([128, C], BF16, tag="w2a")
            nc.sync.dma_start(w2a, w2a_d)
            w2b = persist.tile([CC - 128, C], BF16, tag="w2b")
            nc.sync.dma_start(w2b, w2b_d)
            for q in range(1, NMB):
                for ci in range(NCT):
                    tq = persist.tile([128, MB], BF16, tag=f"x{q}_{ci}")
                    nc.sync.dma_start(
                        tq, x_d[128 * ci : 128 * (ci + 1), MB * q : MB * (q + 1)]
                    )
                    xt[q][ci] = tq

            tri_mask = mi[:, 0:128]
            ident = mi[:, 128:256]

            # v: key-major, interleaved [tile(16), head(3), 64 v + 1 one]
            v_sb = persist.tile([128, NTK * NH_CORE * VW], BF16, tag="v_sb")
            ones_ap = v_sb.rearrange(
                "p (t h u) -> p t h u", t=NTK, h=NH_CORE
            )[:, :, :, D : D + 1]
            nc.gpsimd.memset(ones_ap, 1.0)

            # separate q/k per head (matmul needs equal base partitions);
            # q rows pre-scaled by 1/sqrt(D) host-side
            qT = [
                persist.tile([D, T], BF16, tag=f"qT{hh}", name=f"qT{hh}")
                for hh in range(NH_CORE)
            ]
            kT = [
                persist.tile([D, T], BF16, tag=f"kT{hh}", name=f"kT{hh}")
                for hh in range(NH_CORE)
            ]
            oT_a = persist.tile([128, T], BF16, tag="oT_a")  # heads 0,1
            oT_b = persist.tile([D, T], BF16, tag="oT_b")    # head 2

            def outproj(i):
                for fc in range(C // 128):
                    ps = proj_ps.tile([128, MB], F32, tag="ps")
                    nc.tensor.matmul(
                        ps,
                        lhsT=w2a[:, 128 * fc : 128 * (fc + 1)],
                        rhs=oT_a[:, MB * i : MB * (i + 1)],
                        start=True,
                        stop=False,
                    )
                    nc.tensor.matmul(
                        ps,
                        lhsT=w2b[:, 128 * fc : 128 * (fc + 1)],
                        rhs=oT_b[:, MB * i : MB * (i + 1)],
                        start=False,
                        stop=True,
                    )
                    osb = ostage.tile([128, MB], BF16, tag="osb")
                    nc.vector.tensor_copy(osb, ps)
                    nc.sync.dma_start(
                        out_d[128 * fc : 128 * (fc + 1), MB * i : MB * (i + 1)],
                        osb,
                    )

            for i in range(NMB):
                # ---- qk projection for t-quarter i ----
                for hh in range(NH_CORE):
                    ps = proj_ps.tile([128, MB], F32, tag="ps")
                    for ci in range(NCT):
                        nc.tensor.matmul(
                            ps,
                            lhsT=w1qk[ci][:, 128 * hh : 128 * (hh + 1)],
                            rhs=xt[i][ci],
                            start=(ci == 0),
                            stop=(ci == NCT - 1),
                        )
                    # q bias folded into the copy; k bias is softmax-invariant
                    # (adds a per-query constant to every logit) so dropped.
                    nc.vector.tensor_scalar_add(
                        qT[hh][:, MB * i : MB * (i + 1)],
                        ps[0:D, :],
                        bqk[0:D, hh : hh + 1],
                    )
                    nc.vector.tensor_copy(
                        kT[hh][:, MB * i : MB * (i + 1)], ps[D:128, :]
                    )
                # ---- v projection for t-chunks 4i..4i+3 ----
                for tch in range(4):
                    ps = proj_ps.tile([128, CC], F32, tag="ps")
                    for ci in range(NCT):
                        nc.tensor.matmul(
                            ps,
                            lhsT=xt[i][ci][:, 128 * tch : 128 * (tch + 1)],
                            rhs=w1v[ci],
                            start=(ci == 0),
                            stop=(ci == NCT - 1),
                        )
                    chunk = v_sb[
                        :,
                        NH_CORE * VW * (4 * i + tch) :
                        NH_CORE * VW * (4 * i + tch + 1),
                    ].rearrange("p (h u) -> p h u", h=NH_CORE)
                    nc.vector.tensor_copy(
                        chunk[:, :, 0:D],
                        ps.rearrange("p (h u) -> p h u", h=NH_CORE),
                    )

                # ---- out-projection for the previous m-block ----
                if i > 0:
                    outproj(i - 1)

                # ---- attention for m-block i ----
                def vj_ap(j, hh):
                    base = NH_CORE * VW * j + VW * hh
                    return v_sb[:, base : base + VW]

                for hh in range(NH_CORE):
                    pvps = pv_ps.tile([VW, MB], F32, tag="pv")
                    qrow = qT[hh]
                    krow = kT[hh]
                    # off-diagonal key tiles, two per PSUM tile
                    for j0 in range(0, 4 * i, 2):
                        qkps = qk_ps.tile([128, 2 * MB], F32, tag="qk")
                        for u in range(2):
                            j = j0 + u
                            nc.tensor.matmul(
                                qkps[:, MB * u : MB * (u + 1)],
                                lhsT=krow[:, 128 * j : 128 * (j + 1)],
                                rhs=qrow[:, MB * i : MB * (i + 1)],
                                start=True,
                                stop=True,
                            )
                        pt = pt_pool.tile([128, 2 * MB], BF16, tag="pt")
                        nc.scalar.activation(
                            pt, qkps, mybir.ActivationFunctionType.Exp
                        )
                        for u in range(2):
                            nc.tensor.matmul(
                                pvps,
                                lhsT=vj_ap(j0 + u, hh),
                                rhs=pt[:, MB * u : MB * (u + 1)],
                                start=(j0 + u == 0),
                                stop=False,
                                skip_group_check=True,
                            )
                    # diagonal key tiles p=0..3, narrowed to cols >= 128p,
                    # packed (p0,p1) -> tile A, (p2,p3) -> tile B
                    for pg in range(2):
                        widths = [MB - 128 * (2 * pg), MB - 128 * (2 * pg + 1)]
                        qkps = qk_ps.tile([128, 2 * MB], F32, tag="qk")
                        off = 0
                        for u in range(2):
                            p = 2 * pg + u
                            j = 4 * i + p
                            w = widths[u]
                            nc.tensor.matmul(
                                qkps[:, off : off + w],
                                lhsT=krow[:, 128 * j : 128 * (j + 1)],
                                rhs=qrow[:, MB * i + 128 * p : MB * (i + 1)],
                                start=True,
                                stop=False,
                                skip_group_check=True,
                            )
                            nc.tensor.matmul(
                                qkps[:, off : off + 128],
                                lhsT=ident,
                                rhs=tri_mask,
                                start=False,
                                stop=True,
                                skip_group_check=True,
                            )
                            off += w
                        pt = pt_pool.tile([128, 2 * MB], BF16, tag="pt")
                        nc.scalar.activation(
                            pt[:, 0:off], qkps[:, 0:off],
                            mybir.ActivationFunctionType.Exp,
                        )
                        off = 0
                        for u in range(2):
                            p = 2 * pg + u
                            j = 4 * i + p
                            w = widths[u]
                            nc.tensor.matmul(
                                pvps[:, 128 * p : MB],
                                lhsT=vj_ap(j, hh),
                                rhs=pt[:, off : off + w],
                                start=(i == 0 and p == 0),
                                stop=(p == 3),
                                skip_group_check=True,
                            )
                            off += w
                    # normalize: o = pv[0:64] / pv[64]
                    # (reciprocal_approx_fast is a custom DVE op; stage the
                    # denominator through SBUF before it)
                    drow = small.tile([1, MB], F32, tag="drow")
                    nc.vector.tensor_copy(drow, pvps[D : D + 1, :])
                    rrow = small.tile([1, MB], F32, tag="rrow")
                    nc.vector.reciprocal_approx_fast(rrow, drow)
                    rbc = small.tile([D, MB], F32, tag="rbc")
                    nc.gpsimd.partition_broadcast(rbc, rrow)
                    if hh < 2:
                        odst = oT_a[D * hh : D * (hh + 1), MB * i : MB * (i + 1)]
                    else:
                        odst = oT_b[:, MB * i : MB * (i + 1)]
                    nc.vector.tensor_mul(odst, pvps[0:D, :], rbc)

            outproj(NMB - 1)
            if debug:
                nc.sync.dma_start(v_dbg, v_sb)
                nc.sync.dma_start(q_dbg, qT[0])
                nc.sync.dma_start(k_dbg, kT[0])
                nc.sync.dma_start(oa_dbg, oT_a)
    nc.compile()
    return nc


GROUPS_HEADS = [[3 * g + k for k in range(NH_CORE)] for g in range(GROUPS)]


def _prep_core_inputs(x, W1, b1, W2):
    """Per-core input dicts. Core index = 4*b + g."""
    mi = _build_masks()
    scale = np.float32(1.0 / np.sqrt(D))  # 1/8, exact in fp32
    bf = ml_dtypes.bfloat16
    in_maps = []
    per_g = []
    for g in range(GROUPS):
        heads = GROUPS_HEADS[g]
        w1qk = np.empty((C, 2 * CC), np.float32)
        w1v = np.empty((C, CC), np.float32)
        bqk = np.zeros((128, NH_CORE), np.float32)
        w2T = np.empty((CC, C), np.float32)
        for hh, h in enumerate(heads):
            w1qk[:, 128 * hh : 128 * hh + D] = (W1[D * h : D * h + D] * scale).T
            w1qk[:, 128 * hh + D : 128 * (hh + 1)] = W1[C + D * h : C + D * h + D].T
            w1v[:, D * hh : D * (hh + 1)] = W1[2 * C + D * h : 2 * C + D * h + D].T
            bqk[0:D, hh] = b1[D * h : D * h + D] * scale
            bqk[D:128, hh] = b1[C + D * h : C + D * h + D]
            w2T[D * hh : D * hh + D] = W2[:, D * h : D * h + D].T
        per_g.append(
            {
                "w1qk": w1qk.astype(bf),
                "w1v": w1v.astype(bf),
                "w2a": np.ascontiguousarray(w2T[0:128]).astype(bf),
                "w2b": np.ascontiguousarray(w2T[128:CC]).astype(bf),
                "bqk": bqk,
                "mi": mi,
            }
        )
    for b in range(B):
        xT = np.ascontiguousarray(np.asarray(x[b]).T).astype(bf)  # (768, 2048)
        for g in range(GROUPS):
            in_maps.append({"x": xT, **per_g[g]})
    return in_maps


_PROGRAM_CACHE = {}


def kernel(x, W1, b1, W2, b2):
    global LAST_RESULTS
    x = np.asarray(x, np.float32)
    W1 = np.asarray(W1, np.float32)
    b1 = np.asarray(b1, np.float32)
    W2 = np.asarray(W2, np.float32)
    b2 = np.asarray(b2, np.float32)

    if "prog" not in _PROGRAM_CACHE:
        _PROGRAM_CACHE["prog"] = _build_program()
    nc = _PROGRAM_CACHE["prog"]

    in_maps = _prep_core_inputs(x, W1, b1, W2)
    trace = os.environ.get("KERNEL_TRACE", "0") == "1"
    res = bass_utils.run_bass_kernel_spmd(
        nc, in_maps, core_ids=list(range(NCORES)), trace=trace
    )
    LAST_RESULTS = res

    # v-bias is linear through attention (softmax weights sum to 1):
    # fold it into the output bias.
    b2p = b2 + W2 @ b1[2 * C : 3 * C]
    out = np.empty((B, T, C), np.float32)
    for b in range(B):
        acc = res.results[GROUPS * b]["outT"].astype(np.float32)
        for g in range(1, GROUPS):
            acc = acc + res.results[GROUPS * b + g]["outT"].astype(np.float32)
        out[b] = acc.T + b2p[None, :]
    return out
